# revision 3
# baseline (speedup 1.0000x reference)
"""DiceLoss kernel for Trainium2 (8 NeuronCores, SPMD b-half sharding).

x: (4, 8, 64, 256, 256) f32 logits; target: (4, 1, 64, 256, 256) int labels.
loss = 1 - mean_b mean_c (2*inter[b,c]+1)/(psum[b,c]+tsum[b,c]+1)
  with p = softmax(x, axis=1) flattened over spatial S:
  inter[b,c] = sum_s p[b,c,s]*[t==c], psum = sum_s p, tsum = count(t==c).

Sharding: core i handles batch b = i//2, spatial half i%2 (8 shards =
4 b x 2 halves). Each core accumulates its (psum, inter, tsum) partials
in 6 PSUM banks across ALL 16 chunks with no mid-run flush (the per-b
flush/restart stalls of the S/8-sharded variant disappear), then does a
final on-device free-axis reduction to a [128, 8] f32 tile (4 KB out vs
6 MB of raw PSUM banks). Host combines the 8 tiny outputs in f64.

Per-core device pipeline, chunked [128 part, 8 class, 1024 free],
engine-balanced from measured per-op rates (scalar 9.2us, vector
10.5us, tensor ~10us per chunk vs 13.2us DMA at the 358 GB/s HBM
share -> DMA-saturated steady state):
  SWDGE DMA: x f32->f16 cast, target int->f16 cast
  ScalarE: E = exp(x) fp16; lse = ln(denom); r = exp(-lse) fp16
  TensorE: denom via identity-matmul PSUM accumulation (contiguous rhs)
  VectorE: u8 = E * r broadcast tensor_tensor
           m_c = (T==c) tensor_scalar (4x mode)
           w8 = u8*m8 one broadcast-free tensor_tensor (2x mode)
  TensorE: psum/inter/tsum = ones32-matmuls over u8/w8/m8 slices,
           per-class [32-row, 512] PSUM regions (col-group placement)
  VectorE: final tensor_reduce of each acc bank [128,512] -> [128,1]
"""

import sys
import time

import numpy as np

for _p in ("/opt/trn_rl_repo",):
    if _p not in sys.path:
        sys.path.insert(0, _p)

B = 4
C = 8
S = 64 * 256 * 256  # 4,194,304 spatial positions per (b, c)
NCORES = 8
SC2 = S // 2        # 2,097,152 positions per core (half of one b)
P = 128
F = 1024            # free positions per partition per chunk
CHUNK = P * F       # 131,072 positions per chunk
KCH = SC2 // CHUNK  # 16 chunks per core
SMOOTH = 1.0

PROFILE = False
RUN_KWARGS = {}
LAST = {}

_cache = {}


def _pin_act_tables():
    """Make natural_log_exp_and_others the only table providing Exp/Ln so
    the table-load pass emits one load instead of thrashing between the
    exp-only and ln-only sets. List positions are preserved (walrus maps
    sets by index)."""
    import concourse.bacc as bacc_mod
    from concourse import mybir

    orig = bacc_mod.get_activation_tables

    def patched(arch):
        tables = dict(orig(arch))
        exp = mybir.ActivationFunctionType.Exp
        ln = mybir.ActivationFunctionType.Ln
        for name, funcs in tables.items():
            if name != "natural_log_exp_and_others" and (
                exp in funcs or ln in funcs
            ):
                tables[name] = funcs - {exp, ln}
        return tables

    bacc_mod.get_activation_tables = patched
    return lambda: setattr(bacc_mod, "get_activation_tables", orig)


def _build(tgt_words):
    """Build + compile the Bass program. tgt_words=2 for int64 targets
    (int32 lo/hi pairs), 1 for int32 targets."""
    import concourse.bacc as bacc
    import concourse.tile as tile
    from concourse import mybir

    f32 = mybir.dt.float32
    f16 = mybir.dt.float16
    i32 = mybir.dt.int32
    Alu = mybir.AluOpType
    Act = mybir.ActivationFunctionType
    Ax = mybir.AxisListType

    restore = _pin_act_tables()
    try:
        nc = bacc.Bacc("TRN2", target_bir_lowering=False)

        x_in = nc.dram_tensor("x", [C, SC2], f32, kind="ExternalInput")
        if tgt_words == 2:
            t_in = nc.dram_tensor("t", [SC2, 2], i32, kind="ExternalInput")
        else:
            t_in = nc.dram_tensor("t", [SC2], i32, kind="ExternalInput")
        # col j = stat*2 + bank: ps0 ps1 in0 in1 tp0 tp1 (+2 pad);
        # class c lives at row 32*(c%4) of bank c//4
        o_red = nc.dram_tensor("o", [P, 8], f32, kind="ExternalOutput")

        xv = x_in[:].rearrange("c (k p f) -> k p c f", p=P, f=F)
        if tgt_words == 2:
            tv = t_in[:].rearrange("(k p f) w -> k p f w", p=P, f=F)
        else:
            tv = t_in[:].rearrange("(k p f) -> k p f", p=P, f=F)

        with tile.TileContext(nc) as tc:
            with (
                tc.tile_pool(name="const", bufs=1) as cpool,
                tc.tile_pool(name="xbuf", bufs=3) as xpool,
                tc.tile_pool(name="ebuf", bufs=2) as epool,
                tc.tile_pool(name="ubuf", bufs=2) as upool,
                tc.tile_pool(name="mbuf", bufs=2) as mpool,
                tc.tile_pool(name="wbuf", bufs=2) as wpool,
                tc.tile_pool(name="small", bufs=2) as spool,
                tc.tile_pool(name="psum", bufs=2, space="PSUM") as ppool,
                tc.tile_pool(name="acc", bufs=1, space="PSUM") as apool,
            ):
                ident_i = cpool.tile([P, P], i32)
                nc.gpsimd.iota(ident_i[:], [[1, P]], channel_multiplier=-1)
                ident = cpool.tile([P, P], f16)
                nc.vector.tensor_scalar(
                    out=ident[:], in0=ident_i[:], scalar1=0, scalar2=None,
                    op0=Alu.is_equal,
                )
                ones32 = cpool.tile([P, 32], f16)
                nc.vector.memset(ones32[:], 1.0)

                acc_ps0 = apool.tile([P, 512], f32, tag="acc_ps0")
                acc_ps1 = apool.tile([P, 512], f32, tag="acc_ps1")
                acc_ps = [acc_ps0, acc_ps1]
                acc_in0 = apool.tile([P, 512], f32, tag="acc_in0")
                acc_in1 = apool.tile([P, 512], f32, tag="acc_in1")
                acc_in = [acc_in0, acc_in1]
                acc_tp0 = apool.tile([P, 512], f32, tag="acc_tp0")
                acc_tp1 = apool.tile([P, 512], f32, tag="acc_tp1")
                acc_tp = [acc_tp0, acc_tp1]

                for kk in range(KCH):
                    first = kk == 0
                    last = kk == KCH - 1

                    # small target DMA first: the SWDGE descriptor
                    # queue is serial, and masks (VectorE's earliest
                    # work) only need t16
                    t16 = spool.tile([P, F], f16, tag="t16")
                    if tgt_words == 2:
                        nc.gpsimd.dma_start(out=t16[:],
                                            in_=tv[kk, :, :, 0])
                    else:
                        nc.gpsimd.dma_start(out=t16[:], in_=tv[kk])

                    xtA = xpool.tile([P, C // 2, F], f16, tag="xtA")
                    nc.gpsimd.dma_start(out=xtA[:],
                                        in_=xv[kk][:, 0:4, :])
                    xtB = xpool.tile([P, C // 2, F], f16, tag="xtB")
                    nc.gpsimd.dma_start(out=xtB[:],
                                        in_=xv[kk][:, 4:8, :])

                    e16A = epool.tile([P, C // 2, F], f16, tag="e16A")
                    nc.scalar.activation(e16A[:], xtA[:], Act.Exp)
                    e16B = epool.tile([P, C // 2, F], f16, tag="e16B")
                    nc.scalar.activation(e16B[:], xtB[:], Act.Exp)
                    e_half = [e16A, e16B]

                    r16 = spool.tile([P, F], f16, tag="r16")
                    for h in range(F // 512):
                        sl = slice(h * 512, (h + 1) * 512)
                        dps = ppool.tile([P, 512], f32, tag="dps")
                        for c in range(C):
                            nc.tensor.matmul(
                                dps[:], ident[:],
                                e_half[c // 4][:, c % 4, sl],
                                start=(c == 0), stop=(c == C - 1),
                            )
                        lse = spool.tile([P, 512], f32, tag="lse")
                        nc.scalar.activation(lse[:], dps[:], Act.Ln)
                        nc.scalar.activation(r16[:, sl], lse[:], Act.Exp,
                                             scale=-1.0)

                    # u = (E * 1.0) * r via scalar_tensor_tensor: the
                    # 3-input TensorScalarPtr form runs in DVE 4x_2p
                    # mode (plain tensor_tensor caps at 2x)
                    u8 = upool.tile([P, C, F], f16, tag="u8")
                    rb = r16[:, None, :].broadcast_to((P, C // 2, F))
                    nc.vector.scalar_tensor_tensor(
                        out=u8[:, 0:4, :], in0=e16A[:], scalar=1.0,
                        in1=rb, op0=Alu.mult, op1=Alu.mult)
                    nc.vector.scalar_tensor_tensor(
                        out=u8[:, 4:8, :], in0=e16B[:], scalar=1.0,
                        in1=rb, op0=Alu.mult, op1=Alu.mult)
                    # masks (4x tensor_scalar)
                    m8 = mpool.tile([P, C, F], f16, tag="m8")
                    for c in range(C):
                        nc.vector.tensor_scalar(
                            out=m8[:, c], in0=t16[:], scalar1=float(c),
                            scalar2=None, op0=Alu.is_equal,
                        )
                    # w = (u * 1.0) * m, one 8-class stt op at 4x
                    w8 = wpool.tile([P, C, F], f16, tag="w8")
                    nc.vector.scalar_tensor_tensor(
                        out=w8[:], in0=u8[:], scalar=1.0, in1=m8[:],
                        op0=Alu.mult, op1=Alu.mult)

                    # psum-red: ones32 matmuls over u8
                    for c in range(C):
                        row = 32 * (c % 4)
                        tp = (0, row) if row == 96 else None
                        for h in range(F // 512):
                            sl = slice(h * 512, (h + 1) * 512)
                            nc.tensor.matmul(
                                acc_ps[c // 4][row:row + 32, :],
                                ones32[:], u8[:, c, sl],
                                start=(first and h == 0),
                                stop=(last and h == F // 512 - 1),
                                tile_position=tp,
                                skip_group_check=True,
                            )
                    # inter-red: ones32 matmuls over w8
                    for c in range(C):
                        row = 32 * (c % 4)
                        tp = (0, row) if row == 96 else None
                        for h in range(F // 512):
                            sl = slice(h * 512, (h + 1) * 512)
                            nc.tensor.matmul(
                                acc_in[c // 4][row:row + 32, :],
                                ones32[:], w8[:, c, sl],
                                start=(first and h == 0),
                                stop=(last and h == F // 512 - 1),
                                tile_position=tp,
                                skip_group_check=True,
                            )
                    # tsum-red: ones32 matmuls over m8
                    for c in range(C):
                        row = 32 * (c % 4)
                        tp = (0, row) if row == 96 else None
                        for h in range(F // 512):
                            sl = slice(h * 512, (h + 1) * 512)
                            nc.tensor.matmul(
                                acc_tp[c // 4][row:row + 32, :],
                                ones32[:], m8[:, c, sl],
                                start=(first and h == 0),
                                stop=(last and h == F // 512 - 1),
                                tile_position=tp,
                                skip_group_check=True,
                            )

                # final on-device reduction: [128,512] banks -> [128,8]
                res = spool.tile([P, 8], f32, tag="res")
                for j, acc in enumerate(
                    [acc_ps0, acc_ps1, acc_in0, acc_in1, acc_tp0, acc_tp1]
                ):
                    nc.vector.tensor_reduce(
                        out=res[:, j:j + 1], in_=acc[:],
                        axis=Ax.X, op=Alu.add,
                    )
                nc.vector.memset(res[:, 6:8], 0.0)
                nc.sync.dma_start(out=o_red[:], in_=res[:])

        nc.compile()
    finally:
        restore()
    return nc


def kernel(x, target):
    x = np.asarray(x)
    target = np.asarray(target)
    assert x.shape == (B, C, 64, 256, 256) and x.dtype == np.float32
    tgt_words = 2 if target.dtype == np.int64 else 1

    if tgt_words not in _cache:
        _cache[tgt_words] = _build(tgt_words)
    nc = _cache[tgt_words]

    xr = x.reshape(B, C, S)
    tr = target.reshape(B, S)

    in_maps = []
    for i in range(NCORES):
        b = i // 2
        sl = slice((i % 2) * SC2, (i % 2 + 1) * SC2)
        xs = np.ascontiguousarray(xr[b, :, sl])
        ts = np.ascontiguousarray(tr[b, sl])
        if tgt_words == 2:
            ts = ts.view(np.int32).reshape(SC2, 2)
        else:
            ts = ts.astype(np.int32, copy=False)
        in_maps.append({"x": xs, "t": ts})

    from concourse.bass_utils import run_bass_kernel_spmd

    t0 = time.perf_counter()
    res = run_bass_kernel_spmd(
        nc, in_maps, list(range(NCORES)), trace=PROFILE, **RUN_KWARGS,
    )
    t1 = time.perf_counter()
    LAST["wall_s"] = t1 - t0
    LAST["exec_time_ns"] = res.exec_time_ns
    LAST["mean_exec_time_ns"] = res.mean_exec_time_ns

    ps = np.zeros((B, C), np.float64)
    it = np.zeros((B, C), np.float64)
    tsm = np.zeros((B, C), np.float64)
    for i, r in enumerate(res.results):
        b = i // 2
        o = r["o"].astype(np.float64)  # [P, 8]
        for c in range(C):
            row = 32 * (c % 4)
            bank = c // 4
            ps[b, c] += o[row, 0 + bank]
            it[b, c] += o[row, 2 + bank]
            tsm[b, c] += o[row, 4 + bank]

    dice = (2.0 * it + SMOOTH) / (ps + tsm + SMOOTH)
    loss = 1.0 - dice.mean(axis=1).mean(axis=0)
    return np.float32(loss)
